# revision 23
# baseline (speedup 1.0000x reference)
"""Trainium2 Bass kernel for nn_DiTBlock (B=4,N=1024,C=1024,H=16).

8-way SPMD: core i handles batch i//2, row-half i%2 (512 rows). Each core
computes its batch's full-sequence k/v itself (no collectives). All matmul
activations are kept feature-major (transposed); scores are computed as
S^T [keys, rows] so the additive mask folds into the exp bias (per
partition) and the softmax denominator rides along as a ones-column
appended to V. fp32r for main matmuls, bf16 for attention internals.
"""
import numpy as np
from contextlib import ExitStack

import concourse.bass as bass
import concourse.bacc as bacc
import concourse.mybir as mybir
import concourse.tile as tile
from concourse.bass_utils import run_bass_kernel_spmd
from concourse.masks import make_identity

F32 = mybir.dt.float32
F32R = mybir.dt.float32r
BF16 = mybir.dt.bfloat16
AF = mybir.ActivationFunctionType
ALU = mybir.AluOpType

B, N, C, H, D = 4, 1024, 1024, 16, 64
HID = 4 * C
R = 512            # own rows per core
KT = C // 128      # 8
EPS = 1e-6

_cache = {}


def build_program(debug=False):
    nc = bacc.Bacc(None, target_bir_lowering=False)
    dbg = {}

    def dbg_out(name, ap, shape):
        if not debug:
            return
        t = nc.dram_tensor(name, list(shape), mybir.dt.float32, kind="ExternalOutput")
        out_ap = t[tuple(slice(0, s_) for s_ in shape)]
        if ap.dtype == F32R:
            nc.sync.dma_start(out=out_ap, in_=ap.bitcast(F32))
        elif ap.dtype == BF16:
            nc.gpsimd.dma_start(out=out_ap, in_=ap)
        else:
            nc.sync.dma_start(out=out_ap, in_=ap)
        dbg[name] = t

    x_own = nc.dram_tensor("x_own", [R, C], F32, kind="ExternalInput")
    x_oth = nc.dram_tensor("x_oth", [R, C], F32, kind="ExternalInput")
    cT_d = nc.dram_tensor("cT", [C, N], BF16, kind="ExternalInput")
    mask_sa = nc.dram_tensor("mask_sa", [128, 8], F32, kind="ExternalInput")
    mask_ca = nc.dram_tensor("mask_ca", [128, 8], F32, kind="ExternalInput")
    w_qkv = nc.dram_tensor("w_qkv", [C, 3 * C], BF16, kind="ExternalInput")
    w_proj = nc.dram_tensor("w_proj", [C, C], BF16, kind="ExternalInput")
    w_caq = nc.dram_tensor("w_caq", [C, C], BF16, kind="ExternalInput")
    w_cak = nc.dram_tensor("w_cak", [C, C], BF16, kind="ExternalInput")
    w_cav = nc.dram_tensor("w_cav", [C, C], BF16, kind="ExternalInput")
    w_caproj = nc.dram_tensor("w_caproj", [C, C], BF16, kind="ExternalInput")
    w_fc1 = nc.dram_tensor("w_fc1", [C, HID], F32R, kind="ExternalInput")
    w_fc2 = nc.dram_tensor("w_fc2", [HID, C], BF16, kind="ExternalInput")
    b_qkvT = nc.dram_tensor("b_qkvT", [128, 24], F32, kind="ExternalInput")
    b_caqT = nc.dram_tensor("b_caqT", [128, 8], F32, kind="ExternalInput")
    b_cakT = nc.dram_tensor("b_cakT", [128, 8], F32, kind="ExternalInput")
    b_cavT = nc.dram_tensor("b_cavT", [128, 8], F32, kind="ExternalInput")
    b_fc1T = nc.dram_tensor("b_fc1T", [128, 32], F32, kind="ExternalInput")
    b_fc2T = nc.dram_tensor("b_fc2T", [128, 8], F32, kind="ExternalInput")
    b_projr = nc.dram_tensor("b_projr", [1, C], F32, kind="ExternalInput")
    b_caprojr = nc.dram_tensor("b_caprojr", [1, C], F32, kind="ExternalInput")
    yT = nc.dram_tensor("yT", [C, R], F32, kind="ExternalOutput")

    with tile.TileContext(nc) as tc, ExitStack() as ctx:
        misc = ctx.enter_context(tc.tile_pool(name="misc", bufs=1))
        pxp = ctx.enter_context(tc.tile_pool(name="pxp", bufs=1))
        lnrmp = ctx.enter_context(tc.tile_pool(name="lnrmp", bufs=4))
        statp = ctx.enter_context(tc.tile_pool(name="statp", bufs=4))
        smp = ctx.enter_context(tc.tile_pool(name="smp", bufs=2))
        expp = ctx.enter_context(tc.tile_pool(name="expp", bufs=3))
        replp = ctx.enter_context(tc.tile_pool(name="replp", bufs=2))
        finp = ctx.enter_context(tc.tile_pool(name="finp", bufs=2))
        wcolp = ctx.enter_context(tc.tile_pool(name="wcolp", bufs=3))
        psmm = ctx.enter_context(tc.tile_pool(name="psmm", bufs=2, space="PSUM"))
        psS = ctx.enter_context(tc.tile_pool(name="psS", bufs=2, space="PSUM"))
        psat = ctx.enter_context(tc.tile_pool(name="psat", bufs=2, space="PSUM"))

        ident32 = misc.tile([128, 128], F32)
        make_identity(nc, ident32)
        ident = misc.tile([128, 128], F32R)
        nc.vector.tensor_copy(ident, ident32)
        ones64 = misc.tile([1, 64], BF16)
        nc.vector.memset(ones64, 1.0)
        eps_b = misc.tile([128, 1], F32)
        nc.gpsimd.memset(eps_b, EPS)
        msk_sa = misc.tile([128, 8], F32)
        nc.sync.dma_start(out=msk_sa, in_=mask_sa[:, :])
        msk_ca = misc.tile([128, 8], F32)
        nc.sync.dma_start(out=msk_ca, in_=mask_ca[:, :])
        bqkvT = misc.tile([128, 24], F32)
        nc.sync.dma_start(out=bqkvT, in_=b_qkvT[:, :])
        bcaqT = misc.tile([128, 8], F32)
        nc.sync.dma_start(out=bcaqT, in_=b_caqT[:, :])
        bcakT = misc.tile([128, 8], F32)
        nc.sync.dma_start(out=bcakT, in_=b_cakT[:, :])
        bcavT = misc.tile([128, 8], F32)
        nc.sync.dma_start(out=bcavT, in_=b_cavT[:, :])
        bfc1T = misc.tile([128, 32], F32)
        nc.sync.dma_start(out=bfc1T, in_=b_fc1T[:, :])
        bfc2T = misc.tile([128, 8], F32)
        nc.sync.dma_start(out=bfc2T, in_=b_fc2T[:, :])

        def bcast_load(dst, src_handle):
            s = src_handle[0:1, :]
            ap = bass.AP(tensor=s.tensor, offset=s.offset, ap=[[0, 128], [1, C]])
            nc.gpsimd.dma_start(out=dst, in_=ap)

        brep_proj = misc.tile([128, C], F32)
        bcast_load(brep_proj, b_projr)
        brep_caproj = misc.tile([128, C], F32)
        bcast_load(brep_caproj, b_caprojr)

        x_sb = pxp.tile([128, 4, C], F32, tag="x")
        for rt in range(4):
            nc.sync.dma_start(out=x_sb[:, rt, :], in_=x_own[rt * 128:(rt + 1) * 128, :])

        def ln_stats(src_ap):
            st = statp.tile([128, 2, 6], F32, tag="st", name="st")
            for sg in range(2):
                nc.vector.bn_stats(out=st[:, sg, :],
                                   in_=src_ap[:, sg * 512:(sg + 1) * 512])
            return st

        def ln_finish(sts, src_aps):
            n = len(src_aps)
            mvs = statp.tile([128, 4, 2], F32, tag="mvs", name="mvs")
            for g, st in enumerate(sts):
                nc.vector.bn_aggr(out=mvs[:, g, :], in_=st)
            # rstd = rsqrt(var+eps), batched (bit-trick seed + 2 Newton steps)
            ve = statp.tile([128, 4], F32, tag="ve", name="ve")
            nc.vector.tensor_scalar_add(ve[:, :n], mvs[:, :n, 1], eps_b)
            iv = statp.tile([128, 4], mybir.dt.int32, tag="iv", name="iv")
            nc.vector.tensor_scalar(iv[:, :n], ve[:, :n].bitcast(mybir.dt.int32), 1,
                                    None, ALU.arith_shift_right)
            nc.vector.tensor_scalar(iv[:, :n], iv[:, :n], -1, 0x5F3759DF,
                                    ALU.mult, ALU.add)
            y = iv.bitcast(F32)
            u = statp.tile([128, 4], F32, tag="u", name="u")
            for _ in range(2):
                nc.vector.tensor_tensor(u[:, :n], y[:, :n], y[:, :n], ALU.mult)
                nc.vector.tensor_tensor(u[:, :n], u[:, :n], ve[:, :n], ALU.mult)
                nc.vector.tensor_scalar(u[:, :n], u[:, :n], -0.5, 1.5, ALU.mult, ALU.add)
                nc.vector.tensor_tensor(y[:, :n], y[:, :n], u[:, :n], ALU.mult)
            outs = []
            for g, src_ap in enumerate(src_aps):
                t = lnrmp.tile([128, C], F32R, tag="lnrm", name="lnt")
                nc.vector.tensor_scalar(t, src_ap, mvs[:, g, 0:1], y[:, g:g + 1],
                                        ALU.subtract, ALU.mult)
                outs.append(t)
            return outs

        def ln_group(src_aps):
            return ln_finish([ln_stats(a) for a in src_aps], src_aps)

        def transpose4(srcs, dst_ap, dt=F32R):
            # srcs: 4 row-tile APs [128, C]; dst_ap [128, 8, 512] slice view
            idt = ident if dt == F32R else ident32
            for ct in range(8):
                tp = psmm.tile([128, 512], dt, tag="mm", name="trp")
                for k in range(4):
                    nc.tensor.transpose(
                        tp[:, k * 128:(k + 1) * 128],
                        srcs[k][:, ct * 128:(ct + 1) * 128],
                        idt,
                    )
                nc.scalar.copy(dst_ap[:, ct, :], tp)

        def attention(qTt, kTt, Vt, msk, bvT, attnTt, tap=False):
            for j in range(8):
                paA = psat.tile([65, 512], F32, tag="at", name="paA")
                paB = psat.tile([65, 512], F32, tag="at", name="paB")
                for ky in range(8):
                    ps_ = psS.tile([128, 1024], F32, tag="S", name="sps")
                    kys = slice(ky * 128, (ky + 1) * 128)
                    nc.tensor.matmul(ps_[:, 0:512], kTt[0:64, j, kys], qTt[0:64, j, :])
                    nc.tensor.matmul(ps_[:, 512:1024], kTt[64:128, j, kys], qTt[64:128, j, :])
                    ex = expp.tile([128, 1024], BF16, tag="expS", name="ex")
                    nc.scalar.activation(ex, ps_, AF.Exp, bias=msk[:, ky:ky + 1], scale=0.125)
                    if tap and j == 0 and ky == 0:
                        dbg_out("d_ex0", ex, (128, 1024))
                    nc.tensor.matmul(paA, Vt[:, ky, 2 * j, :], ex[:, 0:512],
                                     start=(ky == 0), stop=(ky == 7))
                    nc.tensor.matmul(paB, Vt[:, ky, 2 * j + 1, :], ex[:, 512:1024],
                                     start=(ky == 0), stop=(ky == 7))
                for half, pa in ((0, paA), (1, paB)):
                    au = replp.tile([65, 512], F32, tag="au", name="au")
                    nc.vector.tensor_copy(au, pa)
                    den = smp.tile([1, 512], F32, tag="den", name="den")
                    nc.vector.tensor_copy(den, au[64:65, :])
                    rc = smp.tile([1, 512], F32, tag="recip", name="rc")
                    nc.vector.reciprocal_approx_fast(out=rc, in_=den)
                    rcb = smp.tile([1, 512], BF16, tag="rcb", name="rcb")
                    nc.vector.tensor_copy(rcb, rc)
                    rp = psat.tile([64, 512], F32, tag="at", name="rp")
                    nc.tensor.matmul(rp, ones64, rcb)
                    if tap and j == 0 and half == 0:
                        dbg_out("d_paA", au, (65, 512))
                        dbg_out("d_rc0", rc, (1, 512))
                    dst = attnTt[half * 64:(half + 1) * 64, j, :]
                    nc.vector.tensor_tensor(dst, au[0:64, :], rp, ALU.mult)
                    nc.vector.tensor_scalar_add(dst, dst, bvT[half * 64:(half + 1) * 64, j:j + 1])

        def col_block_dma(w_handle, o0, dtype, width=256, n_kt=8):
            wc = wcolp.tile([128, n_kt, width], dtype, tag="wcol", name="wc")
            src = w_handle[:, o0:o0 + width].rearrange("(kt p) o -> p kt o", p=128)
            if dtype != w_handle.dtype:
                nc.gpsimd.dma_start(out=wc, in_=src)
            else:
                nc.sync.dma_start(out=wc, in_=src)
            return wc

        def linearT(w_handle, o0_base, n_ot, rhs_fn, rhs_width, out_fn, dtype=F32R):
            # out^T[o, r] for o-tiles: lhsT = W col-blocks, rhs = rhs_fn(kt)
            for og in range(n_ot // 2):
                wc = col_block_dma(w_handle, o0_base + og * 256, dtype)
                for oi in range(2):
                    ot = og * 2 + oi
                    for kc in range(rhs_width // 512):
                        pq = psmm.tile([128, 512], F32, tag="mm", name="pq")
                        for kt in range(KT):
                            nc.tensor.matmul(pq, wc[:, kt, oi * 128:(oi + 1) * 128],
                                             rhs_fn(kt, kc),
                                             start=(kt == 0), stop=(kt == KT - 1))
                        out_fn(ot, kc, pq)

        with tc.tile_pool(name="qTp", bufs=1) as qTp, \
             tc.tile_pool(name="kTp", bufs=1) as kTp, \
             tc.tile_pool(name="Vp", bufs=1) as Vp, \
             tc.tile_pool(name="attnTp", bufs=1) as attnTp, \
             tc.tile_pool(name="wrowp", bufs=8) as wrowp:

            # ---------- Phase B/C: LN1 over all 1024 rows (key order: own|other) + transpose
            with tc.tile_pool(name="lnT1p", bufs=1) as lnT1p:
                ln1T = lnT1p.tile([128, 8, 1024], BF16, tag="lnT")
                for half in range(2):
                    src_aps = []
                    for rt in range(4):
                        if half == 0:
                            src_aps.append(x_sb[:, rt, :])
                        else:
                            xo = lnrmp.tile([128, C], F32, tag="xoth", bufs=4, name="xo")
                            nc.sync.dma_start(out=xo, in_=x_oth[rt * 128:(rt + 1) * 128, :])
                            src_aps.append(xo)
                    srcs = ln_group(src_aps)
                    transpose4(srcs, ln1T.rearrange("p kt (h r) -> p kt h r", h=2)[:, :, half, :])

                dbg_out("d_ln1T", ln1T, (128, 8, 1024))
                # ---------- Phase D: sa qT, kT, V
                qT = qTp.tile([128, 8, 512], BF16, tag="qT")
                linearT(w_qkv, 0, 8, lambda kt, kc: ln1T[:, kt, 0:512], 512,
                        lambda ot, kc, pq: nc.scalar.activation(
                            qT[:, ot, :], pq, AF.Identity, bias=bqkvT[:, ot:ot + 1]),
                        dtype=BF16)
                kT = kTp.tile([128, 8, 1024], BF16, tag="kT")
                linearT(w_qkv, C, 8, lambda kt, kc: ln1T[:, kt, kc * 512:(kc + 1) * 512], 1024,
                        lambda ot, kc, pq: nc.scalar.activation(
                            kT[:, ot, kc * 512:(kc + 1) * 512], pq, AF.Identity,
                            bias=bqkvT[:, 8 + ot:9 + ot]), dtype=BF16)
                V = Vp.tile([128, 8, 16, 65], BF16, tag="V")
                nc.vector.memset(V[:, :, :, 64:65], 1.0)
                wvs = []
                for kt in range(KT):
                    wv = wrowp.tile([128, 1024], BF16, tag="wrow", name="wv")
                    nc.sync.dma_start(out=wv, in_=w_qkv[kt * 128:(kt + 1) * 128, 2048:3072])
                    wvs.append(wv)
                for ky in range(8):
                    for vc in range(2):
                        pv = psmm.tile([128, 512], F32, tag="mm", name="pv")
                        for kt in range(KT):
                            nc.tensor.matmul(pv, ln1T[:, kt, ky * 128:(ky + 1) * 128],
                                             wvs[kt][:, vc * 512:(vc + 1) * 512],
                                             start=(kt == 0), stop=(kt == KT - 1))
                        nc.vector.tensor_copy(
                            V[:, ky, vc * 8:(vc + 1) * 8, 0:64],
                            pv.rearrange("p (h d) -> p h d", h=8))

            # ---------- ln1T released; ca staging + remaining phases
            with tc.tile_pool(name="cTp", bufs=1) as cTp:
                cT_sb = cTp.tile([128, 8, 1024], BF16, tag="cT")
                for kt in range(KT):
                    nc.sync.dma_start(out=cT_sb[:, kt, :],
                                      in_=cT_d[kt * 128:(kt + 1) * 128, :])

                dbg_out("d_qT", qT, (128, 8, 512))
                dbg_out("d_kT", kT, (128, 8, 1024))
                dbg_out("d_V", V, (128, 8, 16, 65))
                # ---------- Phase E: sa attention
                attnT = attnTp.tile([128, 8, 512], BF16, tag="attnT")
                attention(qT, kT, V, msk_sa, bqkvT[:, 16:24], attnT, tap=debug)

                # ---------- Phase F: sa proj + residual into x_sb
                def proj_residual(attnTt, w_handle, brep, ln_after=None):
                    # ln_after: optional (dstT, dt) -> also LayerNorm+transpose x,
                    # interleaved per row-pair so LN starts before proj finishes.
                    wps = []
                    for ft in range(8):
                        wp = wrowp.tile([128, 1024], BF16, tag="wrow", name="wp")
                        nc.sync.dma_start(out=wp, in_=w_handle[ft * 128:(ft + 1) * 128, :])
                        wps.append(wp)
                    sts = []
                    for rt in range(4):
                        for oc in range(2):
                            pp = psmm.tile([128, 512], F32, tag="mm", name="pp")
                            for ft in range(8):
                                nc.tensor.matmul(pp, attnTt[:, ft, rt * 128:(rt + 1) * 128],
                                                 wps[ft][:, oc * 512:(oc + 1) * 512],
                                                 start=(ft == 0), stop=(ft == 7))
                            xsl = x_sb[:, rt, oc * 512:(oc + 1) * 512]
                            nc.vector.tensor_tensor(xsl, xsl, pp, ALU.add)
                            nc.vector.tensor_tensor(xsl, xsl, brep[:, oc * 512:(oc + 1) * 512], ALU.add)
                        if ln_after is not None:
                            sts.append(ln_stats(x_sb[:, rt, :]))
                    lns = []
                    if ln_after is not None:
                        lns = ln_finish(sts, [x_sb[:, r, :] for r in range(4)])
                    if ln_after is not None and ln_after != "defer":
                        dstT, dt = ln_after
                        transpose4(lns, dstT, dt=dt)
                    return lns

                dbg_out("d_attnT", attnT, (128, 8, 512))
                dbg_out("d_x2", x_sb, (128, 4, 1024))

                # ---------- Phase F+G: sa proj + residual + LN2 (interleaved)
                with tc.tile_pool(name="lnT2p", bufs=1) as lnT2p:
                    ln2T = lnT2p.tile([128, 8, 512], BF16, tag="lnT2")
                    proj_residual(attnT, w_proj, brep_proj, ln_after=(ln2T, F32R))

                    # ---------- Phase H: ca qT (bf16 weights x bf16 ln2T)
                    caqT = qTp.tile([128, 8, 512], BF16, tag="qT", name="caqT")
                    linearT(w_caq, 0, 8, lambda kt, kc: ln2T[:, kt, :], 512,
                            lambda ot, kc, pq: nc.scalar.activation(
                                caqT[:, ot, :], pq, AF.Identity,
                                bias=bcaqT[:, ot:ot + 1]), dtype=BF16)

                # ---------- Phase I: ca kT, ca V from cT
                cakT = kTp.tile([128, 8, 1024], BF16, tag="kT", name="cakT")
                linearT(w_cak, 0, 8, lambda kt, kc: cT_sb[:, kt, kc * 512:(kc + 1) * 512], 1024,
                        lambda ot, kc, pq: nc.scalar.activation(
                            cakT[:, ot, kc * 512:(kc + 1) * 512], pq, AF.Identity,
                            bias=bcakT[:, ot:ot + 1]), dtype=BF16)
                caV = Vp.tile([128, 8, 16, 65], BF16, tag="V", name="caV")
                nc.vector.memset(caV[:, :, :, 64:65], 1.0)
                wcavs = []
                for kt in range(KT):
                    wv = wrowp.tile([128, 1024], BF16, tag="wrow", name="wcav")
                    nc.sync.dma_start(out=wv, in_=w_cav[kt * 128:(kt + 1) * 128, :])
                    wcavs.append(wv)
                for ky in range(8):
                    for vc in range(2):
                        pv = psmm.tile([128, 512], F32, tag="mm", name="pcv")
                        for kt in range(KT):
                            nc.tensor.matmul(pv, cT_sb[:, kt, ky * 128:(ky + 1) * 128],
                                             wcavs[kt][:, vc * 512:(vc + 1) * 512],
                                             start=(kt == 0), stop=(kt == KT - 1))
                        nc.vector.tensor_copy(
                            caV[:, ky, vc * 8:(vc + 1) * 8, 0:64],
                            pv.rearrange("p (h d) -> p h d", h=8))

                # ---------- Phase J: ca attention + ca proj
                dbg_out("d_cakT", cakT, (128, 8, 1024))
                caattnT = attnTp.tile([128, 8, 512], BF16, tag="attnT", name="caattnT")
                attention(caqT, cakT, caV, msk_ca, bcavT, caattnT)
                dbg_out("d_caattnT", caattnT, (128, 8, 512))
                lns3 = proj_residual(caattnT, w_caproj, brep_caproj, ln_after="defer")
                dbg_out("d_x3", x_sb, (128, 4, 1024))

        # ---------- MLP scope
        with tc.tile_pool(name="mlpp", bufs=1) as mlpp, \
             tc.tile_pool(name="wfc2p", bufs=3) as wfc2p:
            ln3T = mlpp.tile([128, 8, 512], F32R, tag="lnT3")
            transpose4(lns3, ln3T)
            x3T = mlpp.tile([128, 8, 512], F32, tag="x3T")
            transpose4([x_sb[:, rt, :] for rt in range(4)], x3T, dt=F32)

            dbg_out("d_ln3T", ln3T, (128, 8, 512))
            h1gT = mlpp.tile([128, 32, 512], BF16, tag="h1gT")
            for og in range(16):
                wc = col_block_dma(w_fc1, og * 256, F32R)
                for oi in range(2):
                    ot = og * 2 + oi
                    pf = psmm.tile([128, 512], F32, tag="mm", name="pf")
                    for kt in range(KT):
                        nc.tensor.matmul(pf, wc[:, kt, oi * 128:(oi + 1) * 128],
                                         ln3T[:, kt, :],
                                         start=(kt == 0), stop=(kt == KT - 1))
                    nc.scalar.activation(h1gT[:, ot, :], pf, AF.Gelu,
                                         bias=bfc1T[:, ot:ot + 1])

            for ot in range(8):
                wf = wfc2p.tile([128, 32, 128], BF16, tag="wfc2", name="wf")
                nc.sync.dma_start(out=wf,
                                  in_=w_fc2[:, ot * 128:(ot + 1) * 128].rearrange(
                                      "(kt p) o -> p kt o", p=128))
                pm = psmm.tile([128, 512], F32, tag="mm", name="pm")
                for kt in range(32):
                    nc.tensor.matmul(pm, wf[:, kt, :], h1gT[:, kt, :],
                                     start=(kt == 0), stop=(kt == 31))
                fin = finp.tile([128, 512], F32, tag="fin", name="fin")
                nc.vector.tensor_tensor(fin, pm, x3T[:, ot, :], ALU.add)
                nc.vector.tensor_scalar_add(fin, fin, bfc2T[:, ot:ot + 1])
                nc.sync.dma_start(out=yT[ot * 128:(ot + 1) * 128, :], in_=fin)

    nc.finalize()
    return nc, dbg


def _prep_inputs(i, x, c, mask, sa_qkv_w, sa_qkv_b, sa_proj_w, sa_proj_b,
                 ca_q_w, ca_q_b, ca_k_w, ca_k_b, ca_v_w, ca_v_b,
                 ca_proj_w, ca_proj_b, fc1_w, fc1_b, fc2_w, fc2_b):
    import ml_dtypes
    b, rh = i // 2, i % 2
    r0, r1 = rh * R, (1 - rh) * R
    f = np.float32
    bf = ml_dtypes.bfloat16
    mb = np.where(mask[b] != 1, -10000.0, 0.0).astype(f)
    mb_sa = np.concatenate([mb[r0:r0 + R], mb[r1:r1 + R]])
    return {
        "x_own": np.ascontiguousarray(x[b, r0:r0 + R]).astype(f),
        "x_oth": np.ascontiguousarray(x[b, r1:r1 + R]).astype(f),
        "cT": np.ascontiguousarray(c[b].T).astype(bf),
        "mask_sa": np.ascontiguousarray(mb_sa.reshape(8, 128).T).astype(f),
        "mask_ca": np.ascontiguousarray(mb.reshape(8, 128).T).astype(f),
        "w_qkv": sa_qkv_w.astype(bf),
        "w_proj": sa_proj_w.astype(bf),
        "w_caq": ca_q_w.astype(bf),
        "w_cak": ca_k_w.astype(bf),
        "w_cav": ca_v_w.astype(bf),
        "w_caproj": ca_proj_w.astype(bf),
        "w_fc1": fc1_w.astype(f),
        "w_fc2": fc2_w.astype(bf),
        "b_qkvT": np.ascontiguousarray(sa_qkv_b.reshape(24, 128).T).astype(f),
        "b_caqT": np.ascontiguousarray(ca_q_b.reshape(8, 128).T).astype(f),
        "b_cakT": np.ascontiguousarray(ca_k_b.reshape(8, 128).T).astype(f),
        "b_cavT": np.ascontiguousarray(ca_v_b.reshape(8, 128).T).astype(f),
        "b_fc1T": np.ascontiguousarray(fc1_b.reshape(32, 128).T).astype(f),
        "b_fc2T": np.ascontiguousarray(fc2_b.reshape(8, 128).T).astype(f),
        "b_projr": sa_proj_b.reshape(1, C).astype(f),
        "b_caprojr": ca_proj_b.reshape(1, C).astype(f),
    }


def kernel(**inputs):
    inputs = {k: np.asarray(v) for k, v in inputs.items()}
    if "prog" not in _cache:
        _cache["prog"] = build_program()[0]
    nc = _cache["prog"]
    in_maps = [_prep_inputs(i, **inputs) for i in range(8)]
    res = run_bass_kernel_spmd(nc, in_maps, core_ids=list(range(8)))
    out = np.empty((B, N, C), np.float32)
    for i in range(8):
        b, rh = i // 2, i % 2
        out[b, rh * R:(rh + 1) * R, :] = res.results[i]["yT"].T
    return out
